# revision 18
# baseline (speedup 1.0000x reference)
"""Trainium2 Bass kernel for nn_Decoder (Bahdanau attention + 1-step GRU + fc).

Data-parallel over batch: 64 batches -> 8 cores x 8 batches. All params
replicated; enc_output/hidden/x sharded on axis 0.

Math per batch b (h0 of the GRU is zeros by construction in the reference):
  q      = hidden[b] @ W2 + b1 + b2                         [D]
  featT  = tanh(W1.T @ enc[b].T + q[:,None])                [D, S]
  score  = V.T @ featT                  (+bV, shift-invariant -> dropped)
  attn   = softmax(score)                                   [S]
  ctx    = attn.T @ enc[b]                                  [E]
  xi     = [ctx, x_b, 1] @ [gk; bias_row] (bias_row folds GRU biases)
  z, r   = sigmoid(xi_z), sigmoid(xi_r)
  hh     = tanh(xi_h + r * gb1_h)
  state  = (1 - z) * hh
  out    = state @ fc_W + fc_b
"""

import os
import threading
from contextlib import ExitStack

import numpy as np

import concourse.bass as bass
import concourse.tile as tile
from concourse import bacc, mybir

N_CORES = 8
B, S, E, D = 64, 2048, 512, 512
BPC = B // N_CORES  # batches per core
F32 = mybir.dt.float32
BF16 = mybir.dt.bfloat16
# Matmul compute dtype: float32 (exact, 4 cyc/row) or float32r (1 cyc/row
# when the moving dim >= 256).
MM_DT = mybir.dt.float32r
# featT tiles feed the score matmul and must be produced rounded by ACT;
# ACT cannot emit float32r, so use bf16 for that matmul when MM_DT is f32r.
SC_DT = F32 if MM_DT is F32 else BF16
# When True, enc is cast to bf16 on the Pool engine and the enc transposes,
# feat matmul and ctx matmul run in bf16 (faster PE transposes via FWL).
TR_BF16 = os.environ.get("KBF16", "0") == "1"
NEG = 4  # e-chunks of 128 in E
ND = 4   # d-chunks of 128 in D
NSC = 4  # s-chunks of 512 in S
KSUB = 4  # 128-subchunks per s-chunk


def _c(ap):
    """Bitcast an AP to the matmul compute dtype."""
    if MM_DT is F32 or ap.dtype == MM_DT:
        return ap
    return ap.bitcast(MM_DT)


def _bcast_ap(handle, parts, free_elems, offset=0):
    """AP that broadcasts a contiguous [free_elems] DRAM row across `parts` partitions."""
    a = handle.ap() if hasattr(handle, "ap") and callable(handle.ap) else handle
    return bass.AP(tensor=a.tensor, offset=offset, ap=[[0, parts], [1, free_elems]])


def build_nc(reps=1):
    nc = bacc.Bacc("TRN2", target_bir_lowering=False, num_devices=N_CORES)

    enc_d = nc.dram_tensor("enc", [BPC, S, E], MM_DT, kind="ExternalInput")
    hT_d = nc.dram_tensor("hT", [128, NEG, BPC], MM_DT, kind="ExternalInput")
    xro_d = nc.dram_tensor("xro", [2, BPC], MM_DT, kind="ExternalInput")
    W1_d = nc.dram_tensor("W1", [E, D], MM_DT, kind="ExternalInput")
    W2_d = nc.dram_tensor("W2", [E, D], MM_DT, kind="ExternalInput")
    VT_d = nc.dram_tensor("VT", [128, ND], SC_DT, kind="ExternalInput")
    bsum_d = nc.dram_tensor("bsum", [128, ND], F32, kind="ExternalInput")
    gk_d = nc.dram_tensor("gk", [514, 3 * D], MM_DT, kind="ExternalInput")
    gb1h_d = nc.dram_tensor("gb1h", [1, D], F32, kind="ExternalInput")
    fcW_d = nc.dram_tensor("fcW", [1, D], F32, kind="ExternalInput")
    fcb_d = nc.dram_tensor("fcb", [1, 1], F32, kind="ExternalInput")
    id128_d = nc.dram_tensor("id128", [128, 128], MM_DT, kind="ExternalInput")

    out_d = nc.dram_tensor("out", [BPC, 1], F32, kind="ExternalOutput")
    state_d = nc.dram_tensor("state", [BPC, D], F32, kind="ExternalOutput")
    attn_d = nc.dram_tensor("attn", [BPC, S, 1], F32, kind="ExternalOutput")

    with tile.TileContext(nc) as tc, ExitStack() as ctx:
        _body(ctx, tc, enc_d, hT_d, xro_d, W1_d, W2_d, VT_d, bsum_d, gk_d,
              gb1h_d, fcW_d, fcb_d, id128_d, out_d, state_d, attn_d, reps)
    nc.compile()
    return nc


def _body(ctx, tc, enc_d, hT_d, xro_d, W1_d, W2_d, VT_d, bsum_d, gk_d,
          gb1h_d, fcW_d, fcb_d, id128_d, out_d, state_d, attn_d, reps=1):
    nc = tc.nc
    Tanh = mybir.ActivationFunctionType.Tanh
    Sigmoid = mybir.ActivationFunctionType.Sigmoid
    Exp = mybir.ActivationFunctionType.Exp
    Ident = mybir.ActivationFunctionType.Identity
    X = mybir.AxisListType.X

    sb = ctx.enter_context(tc.tile_pool(name="sb", bufs=1))
    ps = ctx.enter_context(tc.tile_pool(name="ps", bufs=1, space="PSUM"))
    if reps > 1:
        ctx.enter_context(tc.For_i(0, reps, 1))

    def load_enc(b, c):
        bufs = 3 if TR_BF16 else 6
        enc_sb = sb.tile([128, KSUB, E], MM_DT, tag="enc", bufs=bufs,
                         name=f"enc{b}_{c}")
        s0 = c * 512
        nc.sync.dma_start(
            enc_sb[:, 0:2, :],
            enc_d.ap()[b, s0:s0 + 256, :].rearrange("(k p) e -> p k e", p=128))
        nc.scalar.dma_start(
            enc_sb[:, 2:4, :],
            enc_d.ap()[b, s0 + 256:s0 + 512, :].rearrange("(k p) e -> p k e", p=128))
        if not TR_BF16:
            return enc_sb
        enc_bf = sb.tile([128, KSUB, E], BF16, tag="encbf", bufs=6,
                         name=f"encbf{b}_{c}")
        nc.gpsimd.tensor_copy(enc_bf[:, 0:2, :], enc_sb[:, 0:2, :])
        nc.gpsimd.tensor_copy(enc_bf[:, 2:4, :], enc_sb[:, 2:4, :])
        return enc_bf

    enc_pref = {(0, 0): load_enc(0, 0), (0, 1): load_enc(0, 1)}

    # ---- persistent weights / params in SBUF ----
    w1_sb = sb.tile([128, NEG, D], MM_DT, tag="w1")
    w2_sb = sb.tile([128, NEG, D], MM_DT, tag="w2")
    for j in range(NEG):
        nc.sync.dma_start(w2_sb[:, j, :], W2_d.ap()[j * 128:(j + 1) * 128, :])
        nc.sync.dma_start(w1_sb[:, j, :], W1_d.ap()[j * 128:(j + 1) * 128, :])
    gk_sb = sb.tile([128, ND, 3 * D], MM_DT, tag="gk")
    for j in range(ND):
        nc.sync.dma_start(gk_sb[:, j, :], gk_d.ap()[j * 128:(j + 1) * 128, :])
    gk_tail = sb.tile([2, 3 * D], MM_DT, tag="gkt")
    nc.sync.dma_start(gk_tail[:], gk_d.ap()[512:514, :])
    hT_sb = sb.tile([128, NEG, BPC], MM_DT, tag="hT")
    nc.sync.dma_start(hT_sb[:], hT_d[:])
    xro_sb = sb.tile([2, BPC], MM_DT, tag="xro")
    nc.sync.dma_start(xro_sb[:], xro_d[:])
    vT_sb = sb.tile([128, ND], SC_DT, tag="vT")
    nc.sync.dma_start(vT_sb[:], VT_d[:])
    bsum_sb = sb.tile([128, ND], F32, tag="bsum")
    nc.sync.dma_start(bsum_sb[:], bsum_d[:])
    gb1h_bc = sb.tile([BPC, D], F32, tag="gb1h")
    nc.gpsimd.dma_start(out=gb1h_bc[:], in_=_bcast_ap(gb1h_d, BPC, D))
    fcW_bc = sb.tile([BPC, D], F32, tag="fcW")
    nc.gpsimd.dma_start(out=fcW_bc[:], in_=_bcast_ap(fcW_d, BPC, D))
    fcb_bc = sb.tile([BPC, 1], F32, tag="fcb")
    nc.gpsimd.dma_start(out=fcb_bc[:], in_=_bcast_ap(fcb_d, BPC, 1))

    ident128 = sb.tile([128, 128], MM_DT, tag="id128")
    nc.sync.dma_start(ident128[:], id128_d[:])
    id128c = ident128[:]
    id8c = ident128[0:BPC, 0:BPC]
    if TR_BF16:
        w1bf_sb = sb.tile([128, NEG, D], BF16, tag="w1bf")
        for j in range(NEG):
            nc.vector.tensor_copy(w1bf_sb[:, j, :], w1_sb[:, j, :])
        identB = sb.tile([128, 128], BF16, tag="idB")
        nc.vector.tensor_copy(identB[:], ident128[:])
    else:
        w1bf_sb, identB = None, None

    qbT_sb = sb.tile([128, ND, BPC], F32, tag="qbT")

    stage = int(os.environ.get("KSTAGE", "9"))
    if stage < 1:
        return

    # ---- ctx accumulator (partition 0, all batches) ----
    ctx_cat = sb.tile([1, BPC * E], F32, tag="ctxcat")

    # ---- per-batch attention ----
    for b in range(BPC):
        score_sb = sb.tile([1, S], F32, tag="score", bufs=2, name=f"score{b}")
        enc_tiles = []
        for c in range(NSC):
            enc_sb = enc_pref.pop((b, c), None)
            if enc_sb is None:
                enc_sb = load_enc(b, c)
            enc_tiles.append(enc_sb)

            # transpose 512x512 block of enc -> encT tiles [128e, 512s]
            TDT = BF16 if TR_BF16 else MM_DT
            tid = identB if TR_BF16 else id128c
            enct = []
            for j in range(NEG):
                ps_tr = ps.tile([128, 512], TDT, tag="tr", bufs=3,
                                name=f"ps_tr{b}_{c}_{j}")
                for k in range(KSUB):
                    nc.tensor.transpose(
                        ps_tr[:, k * 128:(k + 1) * 128],
                        enc_sb[:, k, j * 128:(j + 1) * 128],
                        tid)
                enct_sb = sb.tile([128, 512], TDT, tag="enct", bufs=6,
                                  name=f"enct{b}_{c}_{j}")
                nc.vector.tensor_copy(enct_sb[:], ps_tr[:])
                enct.append(enct_sb)

            if b == 0 and c == 0:
                # qbT[d, i, b] = (hidden @ W2 + b1 + b2).T for all batches,
                # emitted here so the PE warms up on transposes first
                for i in range(ND):
                    ps_q = ps.tile([128, BPC], F32, tag="sm", bufs=2,
                                   name=f"ps_q{i}")
                    for j in range(NEG):
                        nc.tensor.matmul(
                            ps_q[:], _c(w2_sb[:, j, i * 128:(i + 1) * 128]),
                            _c(hT_sb[:, j, :]),
                            start=(j == 0), stop=(j == NEG - 1))
                    nc.scalar.activation(qbT_sb[:, i, :], ps_q[:], Ident,
                                         bias=bsum_sb[:, i:i + 1])

            # featT[d_i, s] = tanh(W1.T @ encT + q)
            feat = []
            for i in range(ND):
                ps_f = ps.tile([128, 512], F32, tag="feat", bufs=2,
                               name=f"ps_f{b}_{c}_{i}")
                w1t = w1bf_sb if TR_BF16 else w1_sb
                for j in range(NEG):
                    nc.tensor.matmul(
                        ps_f[:], w1t[:, j, i * 128:(i + 1) * 128],
                        enct[j][:],
                        start=(j == 0), stop=(j == NEG - 1))
                f_sb = sb.tile([128, 512], SC_DT, tag="featT", bufs=6,
                               name=f"featT{b}_{c}_{i}")
                nc.scalar.activation(f_sb[:], ps_f[:], Tanh,
                                     bias=qbT_sb[:, i, b:b + 1])
                feat.append(f_sb)

            # score chunk [1, 512]
            ps_sc = ps.tile([1, 512], F32, tag="sm", bufs=2, name=f"ps_sc{b}_{c}")
            for i in range(ND):
                nc.tensor.matmul(ps_sc[:], vT_sb[:, i:i + 1], feat[i][:],
                                 start=(i == 0), stop=(i == ND - 1))
            nc.scalar.copy(score_sb[0:1, c * 512:(c + 1) * 512], ps_sc[:])

        if stage < 2:
            continue
        # softmax over S on partition 0 (in-place exp/normalize)
        nm = sb.tile([1, 1], F32, tag="nm", bufs=2, name=f"nm{b}")
        nc.vector.reduce_max(out=nm[:], in_=score_sb[:], axis=X, negate=True)
        lsum = sb.tile([1, 1], F32, tag="lsum", bufs=2, name=f"lsum{b}")
        nc.scalar.activation(score_sb[:], score_sb[:], Exp, bias=nm[:],
                             accum_out=lsum[:])
        rl = sb.tile([1, 1], F32, tag="rl", bufs=2, name=f"rl{b}")
        nc.vector.reciprocal(rl[:], lsum[:])
        nc.vector.tensor_scalar_mul(score_sb[:], score_sb[:], rl[:])
        nc.sync.dma_start(attn_d.ap()[b, :, :], score_sb[:])

        # attnT [128s, 16] via 16 PE transposes of [1, 128]
        attn_r = sb.tile([NSC * KSUB, 128], MM_DT, tag="attnr", bufs=2,
                         name=f"attnr{b}")
        nc.scalar.dma_start(attn_r[:], _c(score_sb[:]))
        ps_aT = ps.tile([128, NSC * KSUB], MM_DT, tag="sm", bufs=2, name=f"ps_aT{b}")
        nc.tensor.transpose(ps_aT[:], attn_r[:], id128c[0:NSC * KSUB, 0:NSC * KSUB])
        aT_sb = sb.tile([128, NSC * KSUB], BF16 if TR_BF16 else MM_DT,
                        tag="aT", bufs=2, name=f"aT{b}")
        nc.vector.tensor_copy(aT_sb[:], ps_aT[:])

        if stage < 3:
            continue
        # ctx[e] = sum_s attn[s] * enc[s, e]
        ps_ctx = ps.tile([1, E], F32, tag="ctxps", bufs=1, name=f"ps_ctx{b}")
        for c in range(NSC):
            for k in range(KSUB):
                t = c * KSUB + k
                nc.tensor.matmul(ps_ctx[:], aT_sb[:, t:t + 1],
                                 enc_tiles[c][:, k, :],
                                 start=(t == 0), stop=(t == NSC * KSUB - 1))
        nc.scalar.copy(ctx_cat[0:1, b * E:(b + 1) * E], ps_ctx[:])

    if stage < 4:
        return
    # ---- gather ctx to [BPC, E] and transpose to [128, 4, BPC] ----
    ctx_all = sb.tile([BPC, E], MM_DT, tag="ctxall")
    nc.scalar.dma_start(ctx_all[:], _c(ctx_cat[:]))
    ps_cT = ps.tile([128, ND * BPC], MM_DT, tag="sm", bufs=2)
    for k in range(ND):
        nc.tensor.transpose(ps_cT[:, k * BPC:(k + 1) * BPC],
                            ctx_all[:, k * 128:(k + 1) * 128], id8c)
    ctxT_sb = sb.tile([128, ND, BPC], MM_DT, tag="ctxT")
    nc.vector.tensor_copy(ctxT_sb[:], ps_cT[:].rearrange("p (k b) -> p k b", k=ND))

    if stage < 5:
        return
    # ---- GRU gates: xi = [ctx, x, 1] @ [gk; bias_row] ----
    ps_xi = []
    for g in range(3):
        ps_g = ps.tile([BPC, 512], F32, tag="sm", bufs=2, name=f"ps_xi{g}")
        for k in range(ND):
            nc.tensor.matmul(ps_g[:], _c(ctxT_sb[:, k, :]),
                             _c(gk_sb[:, k, g * 512:(g + 1) * 512]),
                             start=(k == 0), stop=False)
        nc.tensor.matmul(ps_g[:], _c(xro_sb[:]),
                         _c(gk_tail[:, g * 512:(g + 1) * 512]),
                         start=False, stop=True)
        ps_xi.append(ps_g)

    if stage < 6:
        return
    z_sb = sb.tile([BPC, 512], F32, tag="z")
    nc.scalar.activation(z_sb[:], ps_xi[0][:], Sigmoid)
    r_sb = sb.tile([BPC, 512], F32, tag="r")
    nc.scalar.activation(r_sb[:], ps_xi[1][:], Sigmoid)
    t1 = sb.tile([BPC, 512], F32, tag="t1")
    nc.vector.tensor_mul(t1[:], r_sb[:], gb1h_bc[:])
    t2 = sb.tile([BPC, 512], F32, tag="t2")
    nc.vector.tensor_add(t2[:], t1[:], ps_xi[2][:])
    hh_sb = sb.tile([BPC, 512], F32, tag="hh")
    nc.scalar.activation(hh_sb[:], t2[:], Tanh)
    omz = sb.tile([BPC, 512], F32, tag="omz")
    nc.scalar.activation(omz[:], z_sb[:], Ident, bias=1.0, scale=-1.0)
    state_sb = sb.tile([BPC, 512], F32, tag="state")
    nc.vector.tensor_mul(state_sb[:], omz[:], hh_sb[:])
    nc.sync.dma_start(state_d[:], state_sb[:])

    if stage < 7:
        return
    # ---- out = state @ fc_W + fc_b ----
    scr = sb.tile([BPC, 512], F32, tag="scr")
    out_sb = sb.tile([BPC, 1], F32, tag="outsb")
    nc.vector.tensor_mul(scr[:], state_sb[:], fcW_bc[:])
    nc.vector.reduce_sum(out=out_sb[:], in_=scr[:], axis=X)
    nc.scalar.activation(out_sb[:], out_sb[:], Ident, bias=fcb_bc[:])
    nc.sync.dma_start(out_d[:], out_sb[:])


_NC = None
_NC_LOCK = threading.Lock()


def get_nc():
    global _NC
    with _NC_LOCK:
        if _NC is None:
            _NC = build_nc()
    return _NC


def make_in_maps(x, hidden, enc_output, W1, b1, W2, b2, V, bV,
                 gru_kernel, gru_rec_kernel, gru_bias, fc_W, fc_b):
    f = np.float32
    x = np.asarray(x, f)
    hidden = np.asarray(hidden, f)
    enc_output = np.ascontiguousarray(np.asarray(enc_output, f))
    W1 = np.ascontiguousarray(np.asarray(W1, f))
    W2 = np.ascontiguousarray(np.asarray(W2, f))
    V = np.asarray(V, f)
    gru_kernel = np.asarray(gru_kernel, f)
    gru_bias = np.asarray(gru_bias, f)
    fc_W = np.asarray(fc_W, f)
    fc_b = np.asarray(fc_b, f)

    # tiny host-side layout prep (all O(KB))
    VT = np.ascontiguousarray(V.reshape(ND, 128).T)                 # [128, 4]
    if MM_DT is not F32:
        import ml_dtypes
        VT = VT.astype(ml_dtypes.bfloat16)
    bsum = np.ascontiguousarray((b1 + b2).astype(f).reshape(ND, 128).T)
    gb0, gb1 = gru_bias[0], gru_bias[1]
    bias_row = np.concatenate([gb0[:1024] + gb1[:1024], gb0[1024:]]).astype(f)
    gk_aug = np.ascontiguousarray(
        np.concatenate([gru_kernel, bias_row[None, :]], axis=0))    # [514, 1536]
    gb1h = np.ascontiguousarray(gb1[1024:][None, :])                # [1, 512]
    fcW_row = np.ascontiguousarray(fc_W.reshape(1, D))
    fcb = np.ascontiguousarray(fc_b.reshape(1, 1))

    in_maps = []
    for cix in range(N_CORES):
        sl = slice(cix * BPC, (cix + 1) * BPC)
        h_sh = hidden[sl]                                           # [8, 512]
        hT = np.ascontiguousarray(h_sh.T.reshape(NEG, 128, BPC).transpose(1, 0, 2))
        xro = np.stack([x[sl, 0, 0], np.ones(BPC, f)])              # [2, 8]
        in_maps.append({
            "id128": np.eye(128, dtype=f),
            "enc": enc_output[sl],
            "hT": hT,
            "xro": np.ascontiguousarray(xro),
            "W1": W1, "W2": W2, "VT": VT, "bsum": bsum,
            "gk": gk_aug, "gb1h": gb1h, "fcW": fcW_row, "fcb": fcb,
        })
    return in_maps


def assemble(results):
    out = np.concatenate([r["out"] for r in results], axis=0)
    state = np.concatenate([r["state"] for r in results], axis=0)
    attn = np.concatenate([r["attn"] for r in results], axis=0)
    return out, state, attn


def kernel(**inputs):
    from concourse.bass_utils import run_bass_kernel_spmd
    nc = get_nc()
    in_maps = make_in_maps(**inputs)
    res = run_bass_kernel_spmd(nc, in_maps, list(range(N_CORES)))
    return assemble(res.results)


# revision 20
# speedup vs baseline: 1.0338x; 1.0338x over previous
"""Trainium2 Bass kernel for nn_Decoder (Bahdanau attention + 1-step GRU + fc).

Data-parallel over batch: 64 batches -> 8 cores x 8 batches. All params
replicated; enc_output/hidden/x sharded on axis 0.

Math per batch b (h0 of the GRU is zeros by construction in the reference):
  q      = hidden[b] @ W2 + b1 + b2                         [D]
  featT  = tanh(W1.T @ enc[b].T + q[:,None])                [D, S]
  score  = V.T @ featT                  (+bV, shift-invariant -> dropped)
  attn   = softmax(score)                                   [S]
  ctx    = attn.T @ enc[b]                                  [E]
  xi     = [ctx, x_b, 1] @ [gk; bias_row] (bias_row folds GRU biases)
  z, r   = sigmoid(xi_z), sigmoid(xi_r)
  hh     = tanh(xi_h + r * gb1_h)
  state  = (1 - z) * hh
  out    = state @ fc_W + fc_b
"""

import os
import threading
from contextlib import ExitStack

import numpy as np

import concourse.bass as bass
import concourse.tile as tile
from concourse import bacc, mybir

N_CORES = 8
B, S, E, D = 64, 2048, 512, 512
BPC = B // N_CORES  # batches per core
F32 = mybir.dt.float32
BF16 = mybir.dt.bfloat16
# Matmul compute dtype: float32 (exact, 4 cyc/row) or float32r (1 cyc/row
# when the moving dim >= 256).
MM_DT = mybir.dt.float32r
# featT tiles feed the score matmul and must be produced rounded by ACT;
# ACT cannot emit float32r, so use bf16 for that matmul when MM_DT is f32r.
SC_DT = F32 if MM_DT is F32 else BF16
# When True, enc is cast to bf16 on the Pool engine and the enc transposes,
# feat matmul and ctx matmul run in bf16 (faster PE transposes via FWL).
TR_BF16 = os.environ.get("KBF16", "0") == "1"
NEG = 4  # e-chunks of 128 in E
ND = 4   # d-chunks of 128 in D
NSC = 4  # s-chunks of 512 in S
KSUB = 4  # 128-subchunks per s-chunk


def _c(ap):
    """Bitcast an AP to the matmul compute dtype."""
    if MM_DT is F32 or ap.dtype == MM_DT:
        return ap
    return ap.bitcast(MM_DT)


def _bcast_ap(handle, parts, free_elems, offset=0):
    """AP that broadcasts a contiguous [free_elems] DRAM row across `parts` partitions."""
    a = handle.ap() if hasattr(handle, "ap") and callable(handle.ap) else handle
    return bass.AP(tensor=a.tensor, offset=offset, ap=[[0, parts], [1, free_elems]])


def build_nc(reps=1):
    nc = bacc.Bacc("TRN2", target_bir_lowering=False, num_devices=N_CORES)

    enc_d = nc.dram_tensor("enc", [BPC, S, E], MM_DT, kind="ExternalInput")
    hT_d = nc.dram_tensor("hT", [128, NEG, BPC], MM_DT, kind="ExternalInput")
    xro_d = nc.dram_tensor("xro", [2, BPC], MM_DT, kind="ExternalInput")
    W1_d = nc.dram_tensor("W1", [E, D], MM_DT, kind="ExternalInput")
    W2_d = nc.dram_tensor("W2", [E, D], MM_DT, kind="ExternalInput")
    VT_d = nc.dram_tensor("VT", [128, ND], SC_DT, kind="ExternalInput")
    bsum_d = nc.dram_tensor("bsum", [128, ND], F32, kind="ExternalInput")
    gk_d = nc.dram_tensor("gk", [514, 3 * D], MM_DT, kind="ExternalInput")
    gb1h_d = nc.dram_tensor("gb1h", [1, D], F32, kind="ExternalInput")
    fcW_d = nc.dram_tensor("fcW", [1, D], F32, kind="ExternalInput")
    fcb_d = nc.dram_tensor("fcb", [1, 1], F32, kind="ExternalInput")
    id128_d = nc.dram_tensor("id128", [128, 128], MM_DT, kind="ExternalInput")

    out_d = nc.dram_tensor("out", [BPC, 1], F32, kind="ExternalOutput")
    state_d = nc.dram_tensor("state", [BPC, D], F32, kind="ExternalOutput")
    attn_d = nc.dram_tensor("attn", [BPC, S, 1], F32, kind="ExternalOutput")

    with tile.TileContext(nc) as tc, ExitStack() as ctx:
        _body(ctx, tc, enc_d, hT_d, xro_d, W1_d, W2_d, VT_d, bsum_d, gk_d,
              gb1h_d, fcW_d, fcb_d, id128_d, out_d, state_d, attn_d, reps)
    nc.compile()
    return nc


def _body(ctx, tc, enc_d, hT_d, xro_d, W1_d, W2_d, VT_d, bsum_d, gk_d,
          gb1h_d, fcW_d, fcb_d, id128_d, out_d, state_d, attn_d, reps=1):
    nc = tc.nc
    Tanh = mybir.ActivationFunctionType.Tanh
    Sigmoid = mybir.ActivationFunctionType.Sigmoid
    Exp = mybir.ActivationFunctionType.Exp
    Ident = mybir.ActivationFunctionType.Identity
    X = mybir.AxisListType.X

    sb = ctx.enter_context(tc.tile_pool(name="sb", bufs=1))
    ps = ctx.enter_context(tc.tile_pool(name="ps", bufs=1, space="PSUM"))
    if reps > 1:
        ctx.enter_context(tc.For_i(0, reps, 1))

    def load_enc(b, c):
        bufs = 3 if TR_BF16 else 8
        enc_sb = sb.tile([128, KSUB, E], MM_DT, tag="enc", bufs=bufs,
                         name=f"enc{b}_{c}")
        s0 = c * 512
        nc.sync.dma_start(
            enc_sb[:, 0:2, :],
            enc_d.ap()[b, s0:s0 + 256, :].rearrange("(k p) e -> p k e", p=128))
        nc.scalar.dma_start(
            enc_sb[:, 2:4, :],
            enc_d.ap()[b, s0 + 256:s0 + 512, :].rearrange("(k p) e -> p k e", p=128))
        if not TR_BF16:
            return enc_sb
        enc_bf = sb.tile([128, KSUB, E], BF16, tag="encbf", bufs=6,
                         name=f"encbf{b}_{c}")
        nc.gpsimd.tensor_copy(enc_bf[:, 0:2, :], enc_sb[:, 0:2, :])
        nc.gpsimd.tensor_copy(enc_bf[:, 2:4, :], enc_sb[:, 2:4, :])
        return enc_bf

    enc_pref = {(0, c): load_enc(0, c) for c in range(3)}

    # ---- persistent weights / params in SBUF ----
    w1_sb = sb.tile([128, NEG, D], MM_DT, tag="w1")
    w2_sb = sb.tile([128, NEG, D], MM_DT, tag="w2")
    for j in range(NEG):
        (nc.sync if j % 2 == 0 else nc.scalar).dma_start(
            w2_sb[:, j, :], W2_d.ap()[j * 128:(j + 1) * 128, :])
        (nc.scalar if j % 2 == 0 else nc.sync).dma_start(
            w1_sb[:, j, :], W1_d.ap()[j * 128:(j + 1) * 128, :])
    hT_sb = sb.tile([128, NEG, BPC], MM_DT, tag="hT")
    nc.scalar.dma_start(hT_sb[:], hT_d[:])
    vT_sb = sb.tile([128, ND], SC_DT, tag="vT")
    nc.scalar.dma_start(vT_sb[:], VT_d[:])
    bsum_sb = sb.tile([128, ND], F32, tag="bsum")
    nc.scalar.dma_start(bsum_sb[:], bsum_d[:])
    # GRU-only params load in the background on the Pool (SWDGE) queue
    gk_sb = sb.tile([128, ND, 3 * D], MM_DT, tag="gk")
    for j in range(ND):
        nc.gpsimd.dma_start(gk_sb[:, j, :], gk_d.ap()[j * 128:(j + 1) * 128, :])
    gk_tail = sb.tile([2, 3 * D], MM_DT, tag="gkt")
    nc.gpsimd.dma_start(gk_tail[:], gk_d.ap()[512:514, :])
    xro_sb = sb.tile([2, BPC], MM_DT, tag="xro")
    nc.gpsimd.dma_start(xro_sb[:], xro_d[:])
    gb1h_bc = sb.tile([BPC, D], F32, tag="gb1h")
    nc.gpsimd.dma_start(out=gb1h_bc[:], in_=_bcast_ap(gb1h_d, BPC, D))
    fcW_bc = sb.tile([BPC, D], F32, tag="fcW")
    nc.gpsimd.dma_start(out=fcW_bc[:], in_=_bcast_ap(fcW_d, BPC, D))
    fcb_bc = sb.tile([BPC, 1], F32, tag="fcb")
    nc.gpsimd.dma_start(out=fcb_bc[:], in_=_bcast_ap(fcb_d, BPC, 1))

    ident128 = sb.tile([128, 128], MM_DT, tag="id128")
    nc.sync.dma_start(ident128[:], id128_d[:])
    id128c = ident128[:]
    id8c = ident128[0:BPC, 0:BPC]
    if TR_BF16:
        w1bf_sb = sb.tile([128, NEG, D], BF16, tag="w1bf")
        for j in range(NEG):
            nc.vector.tensor_copy(w1bf_sb[:, j, :], w1_sb[:, j, :])
        identB = sb.tile([128, 128], BF16, tag="idB")
        nc.vector.tensor_copy(identB[:], ident128[:])
    else:
        w1bf_sb, identB = None, None

    qbT_sb = sb.tile([128, ND, BPC], F32, tag="qbT")

    stage = 9

    # ---- ctx accumulator (partition 0, all batches) ----
    ctx_cat = sb.tile([1, BPC * E], F32, tag="ctxcat")

    # ---- per-batch attention ----
    for b in range(BPC):
        score_sb = sb.tile([1, S], F32, tag="score", bufs=2, name=f"score{b}")
        enc_tiles = []
        for c in range(NSC):
            enc_sb = enc_pref.pop((b, c), None)
            if enc_sb is None:
                enc_sb = load_enc(b, c)
            enc_tiles.append(enc_sb)

            # transpose 512x512 block of enc -> encT tiles [128e, 512s]
            TDT = BF16 if TR_BF16 else MM_DT
            tid = identB if TR_BF16 else id128c
            enct = []
            for j in range(NEG):
                ps_tr = ps.tile([128, 512], TDT, tag="tr", bufs=3,
                                name=f"ps_tr{b}_{c}_{j}")
                for k in range(KSUB):
                    nc.tensor.transpose(
                        ps_tr[:, k * 128:(k + 1) * 128],
                        enc_sb[:, k, j * 128:(j + 1) * 128],
                        tid)
                enct_sb = sb.tile([128, 512], TDT, tag="enct", bufs=6,
                                  name=f"enct{b}_{c}_{j}")
                nc.vector.tensor_copy(enct_sb[:], ps_tr[:])
                enct.append(enct_sb)

            if b == 0 and c == 0:
                # qbT[d, i, b] = (hidden @ W2 + b1 + b2).T for all batches,
                # emitted here so the PE warms up on transposes first
                for i in range(ND):
                    ps_q = ps.tile([128, BPC], F32, tag="sm", bufs=2,
                                   name=f"ps_q{i}")
                    for j in range(NEG):
                        nc.tensor.matmul(
                            ps_q[:], _c(w2_sb[:, j, i * 128:(i + 1) * 128]),
                            _c(hT_sb[:, j, :]),
                            start=(j == 0), stop=(j == NEG - 1))
                    nc.scalar.activation(qbT_sb[:, i, :], ps_q[:], Ident,
                                         bias=bsum_sb[:, i:i + 1])

            # featT[d_i, s] = tanh(W1.T @ encT + q)
            feat = []
            for i in range(ND):
                ps_f = ps.tile([128, 512], F32, tag="feat", bufs=2,
                               name=f"ps_f{b}_{c}_{i}")
                w1t = w1bf_sb if TR_BF16 else w1_sb
                for j in range(NEG):
                    nc.tensor.matmul(
                        ps_f[:], w1t[:, j, i * 128:(i + 1) * 128],
                        enct[j][:],
                        start=(j == 0), stop=(j == NEG - 1))
                f_sb = sb.tile([128, 512], SC_DT, tag="featT", bufs=6,
                               name=f"featT{b}_{c}_{i}")
                nc.scalar.activation(f_sb[:], ps_f[:], Tanh,
                                     bias=qbT_sb[:, i, b:b + 1])
                feat.append(f_sb)

            # score chunk [1, 512]
            ps_sc = ps.tile([1, 512], F32, tag="sm", bufs=2, name=f"ps_sc{b}_{c}")
            for i in range(ND):
                nc.tensor.matmul(ps_sc[:], vT_sb[:, i:i + 1], feat[i][:],
                                 start=(i == 0), stop=(i == ND - 1))
            nc.scalar.copy(score_sb[0:1, c * 512:(c + 1) * 512], ps_sc[:])

        if stage < 2:
            continue
        # softmax over S on partition 0 (in-place exp/normalize)
        nm = sb.tile([1, 1], F32, tag="nm", bufs=2, name=f"nm{b}")
        nc.vector.reduce_max(out=nm[:], in_=score_sb[:], axis=X, negate=True)
        lsum = sb.tile([1, 1], F32, tag="lsum", bufs=2, name=f"lsum{b}")
        nc.scalar.activation(score_sb[:], score_sb[:], Exp, bias=nm[:],
                             accum_out=lsum[:])
        rl = sb.tile([1, 1], F32, tag="rl", bufs=2, name=f"rl{b}")
        nc.vector.reciprocal(rl[:], lsum[:])
        nc.vector.tensor_scalar_mul(score_sb[:], score_sb[:], rl[:])
        nc.sync.dma_start(attn_d.ap()[b, :, :], score_sb[:])

        # attnT [128s, 16] via 16 PE transposes of [1, 128]
        attn_r = sb.tile([NSC * KSUB, 128], MM_DT, tag="attnr", bufs=2,
                         name=f"attnr{b}")
        nc.scalar.dma_start(attn_r[:], _c(score_sb[:]))
        ps_aT = ps.tile([128, NSC * KSUB], MM_DT, tag="sm", bufs=2, name=f"ps_aT{b}")
        nc.tensor.transpose(ps_aT[:], attn_r[:], id128c[0:NSC * KSUB, 0:NSC * KSUB])
        aT_sb = sb.tile([128, NSC * KSUB], BF16 if TR_BF16 else MM_DT,
                        tag="aT", bufs=2, name=f"aT{b}")
        nc.vector.tensor_copy(aT_sb[:], ps_aT[:])

        if stage < 3:
            continue
        # ctx[e] = sum_s attn[s] * enc[s, e]
        ps_ctx = ps.tile([1, E], F32, tag="ctxps", bufs=1, name=f"ps_ctx{b}")
        for c in range(NSC):
            for k in range(KSUB):
                t = c * KSUB + k
                nc.tensor.matmul(ps_ctx[:], aT_sb[:, t:t + 1],
                                 enc_tiles[c][:, k, :],
                                 start=(t == 0), stop=(t == NSC * KSUB - 1))
        nc.scalar.copy(ctx_cat[0:1, b * E:(b + 1) * E], ps_ctx[:])

    if stage < 4:
        return
    # ---- gather ctx to [BPC, E] and transpose to [128, 4, BPC] ----
    ctx_all = sb.tile([BPC, E], MM_DT, tag="ctxall")
    nc.scalar.dma_start(ctx_all[:], _c(ctx_cat[:]))
    ps_cT = ps.tile([128, ND * BPC], MM_DT, tag="sm", bufs=2)
    for k in range(ND):
        nc.tensor.transpose(ps_cT[:, k * BPC:(k + 1) * BPC],
                            ctx_all[:, k * 128:(k + 1) * 128], id8c)
    ctxT_sb = sb.tile([128, ND, BPC], MM_DT, tag="ctxT")
    nc.vector.tensor_copy(ctxT_sb[:], ps_cT[:].rearrange("p (k b) -> p k b", k=ND))

    if stage < 5:
        return
    # ---- GRU gates: xi = [ctx, x, 1] @ [gk; bias_row] ----
    ps_xi = []
    for g in range(3):
        ps_g = ps.tile([BPC, 512], F32, tag="sm", bufs=2, name=f"ps_xi{g}")
        for k in range(ND):
            nc.tensor.matmul(ps_g[:], _c(ctxT_sb[:, k, :]),
                             _c(gk_sb[:, k, g * 512:(g + 1) * 512]),
                             start=(k == 0), stop=False)
        nc.tensor.matmul(ps_g[:], _c(xro_sb[:]),
                         _c(gk_tail[:, g * 512:(g + 1) * 512]),
                         start=False, stop=True)
        ps_xi.append(ps_g)

    if stage < 6:
        return
    z_sb = sb.tile([BPC, 512], F32, tag="z")
    nc.scalar.activation(z_sb[:], ps_xi[0][:], Sigmoid)
    r_sb = sb.tile([BPC, 512], F32, tag="r")
    nc.scalar.activation(r_sb[:], ps_xi[1][:], Sigmoid)
    t1 = sb.tile([BPC, 512], F32, tag="t1")
    nc.vector.tensor_mul(t1[:], r_sb[:], gb1h_bc[:])
    t2 = sb.tile([BPC, 512], F32, tag="t2")
    nc.vector.tensor_add(t2[:], t1[:], ps_xi[2][:])
    hh_sb = sb.tile([BPC, 512], F32, tag="hh")
    nc.scalar.activation(hh_sb[:], t2[:], Tanh)
    omz = sb.tile([BPC, 512], F32, tag="omz")
    nc.scalar.activation(omz[:], z_sb[:], Ident, bias=1.0, scale=-1.0)
    state_sb = sb.tile([BPC, 512], F32, tag="state")
    nc.vector.tensor_mul(state_sb[:], omz[:], hh_sb[:])
    nc.sync.dma_start(state_d[:], state_sb[:])

    if stage < 7:
        return
    # ---- out = state @ fc_W + fc_b ----
    scr = sb.tile([BPC, 512], F32, tag="scr")
    out_sb = sb.tile([BPC, 1], F32, tag="outsb")
    nc.vector.tensor_mul(scr[:], state_sb[:], fcW_bc[:])
    nc.vector.reduce_sum(out=out_sb[:], in_=scr[:], axis=X)
    nc.scalar.activation(out_sb[:], out_sb[:], Ident, bias=fcb_bc[:])
    nc.sync.dma_start(out_d[:], out_sb[:])


_NC = None
_NC_LOCK = threading.Lock()


def get_nc():
    global _NC
    with _NC_LOCK:
        if _NC is None:
            _NC = build_nc()
    return _NC


def make_in_maps(x, hidden, enc_output, W1, b1, W2, b2, V, bV,
                 gru_kernel, gru_rec_kernel, gru_bias, fc_W, fc_b):
    f = np.float32
    x = np.asarray(x, f)
    hidden = np.asarray(hidden, f)
    enc_output = np.ascontiguousarray(np.asarray(enc_output, f))
    W1 = np.ascontiguousarray(np.asarray(W1, f))
    W2 = np.ascontiguousarray(np.asarray(W2, f))
    V = np.asarray(V, f)
    gru_kernel = np.asarray(gru_kernel, f)
    gru_bias = np.asarray(gru_bias, f)
    fc_W = np.asarray(fc_W, f)
    fc_b = np.asarray(fc_b, f)

    # tiny host-side layout prep (all O(KB))
    VT = np.ascontiguousarray(V.reshape(ND, 128).T)                 # [128, 4]
    if MM_DT is not F32:
        import ml_dtypes
        VT = VT.astype(ml_dtypes.bfloat16)
    bsum = np.ascontiguousarray((b1 + b2).astype(f).reshape(ND, 128).T)
    gb0, gb1 = gru_bias[0], gru_bias[1]
    bias_row = np.concatenate([gb0[:1024] + gb1[:1024], gb0[1024:]]).astype(f)
    gk_aug = np.ascontiguousarray(
        np.concatenate([gru_kernel, bias_row[None, :]], axis=0))    # [514, 1536]
    gb1h = np.ascontiguousarray(gb1[1024:][None, :])                # [1, 512]
    fcW_row = np.ascontiguousarray(fc_W.reshape(1, D))
    fcb = np.ascontiguousarray(fc_b.reshape(1, 1))

    in_maps = []
    for cix in range(N_CORES):
        sl = slice(cix * BPC, (cix + 1) * BPC)
        h_sh = hidden[sl]                                           # [8, 512]
        hT = np.ascontiguousarray(h_sh.T.reshape(NEG, 128, BPC).transpose(1, 0, 2))
        xro = np.stack([x[sl, 0, 0], np.ones(BPC, f)])              # [2, 8]
        in_maps.append({
            "id128": np.eye(128, dtype=f),
            "enc": enc_output[sl],
            "hT": hT,
            "xro": np.ascontiguousarray(xro),
            "W1": W1, "W2": W2, "VT": VT, "bsum": bsum,
            "gk": gk_aug, "gb1h": gb1h, "fcW": fcW_row, "fcb": fcb,
        })
    return in_maps


def assemble(results):
    out = np.concatenate([r["out"] for r in results], axis=0)
    state = np.concatenate([r["state"] for r in results], axis=0)
    attn = np.concatenate([r["attn"] for r in results], axis=0)
    return out, state, attn


def kernel(**inputs):
    from concourse.bass_utils import run_bass_kernel_spmd
    nc = get_nc()
    in_maps = make_in_maps(**inputs)
    res = run_bass_kernel_spmd(nc, in_maps, list(range(N_CORES)))
    return assemble(res.results)
